# revision 1
# baseline (speedup 1.0000x reference)
"""Trainium2 Bass kernel for nn_NeuralGraphHidden (GNN message passing).

Key insight: edges ~ randint(-1, 128) gives P(edge == -1) = 1/129, so ~95.5%
of atoms have degree 6 — and the reference's degree mask only covers degrees
0..5, so those atoms' outputs are EXACTLY ZERO.  Only atoms with degree < 6
("active" atoms, ~190 per core) ever contribute to the output, so the message
pipeline only needs their ~1150 edge slots, not all 196k.

The host shards the batch over 8 cores, buckets active atoms by degree
(uniform bucket capacities across cores so a single SPMD program serves all
8), and stages everything pre-transposed (feature-major) so the device never
transposes.  Neighbour atom features are staged per edge slot (cheap at this
sparsity), so the device pipeline is pure matmul + elementwise, per degree
block d:

  pre_d  = W0a.T @ nbrT_d  +  W0b.T @ bondsT_d   (PSUM accumulate)
  msg0_d = elu(pre_d)    elu(x) = min(exp(x),1) + relu(x) - 1  (ACT exp + DVE)
  msg1_d = elu(W1.T @ msg0_d)
  summed = sum_d msg1_d                          (DVE adds, tree)
  h0     = elu(W0d_hi.T @ summed + W0d_lo.T @ actT)    per degree bucket
  out    = elu(h0_chunk.T @ W1d)                 (data-stationary -> atom-major)

Matmul operands are float32r (PE streams fp32 ~2-4x faster than plain
float32); accumulation and elu math stay f32 via PSUM.  Inputs are DMA'd in
dependency order so the first matmuls overlap the remaining loads, and a
short warm-up matmul burst during the DMA wait ramps the PE clock.
The host scatters the few computed rows into the (mostly zero) full output.
"""

import sys

if "/opt/trn_rl_repo" not in sys.path:
    sys.path.insert(0, "/opt/trn_rl_repo")

import numpy as np
import ml_dtypes

import concourse.bass as bass
import concourse.bacc as bacc
import concourse.mybir as mybir
import concourse.tile as tile
from concourse import bass_utils

import concourse.dve_ops as dve_ops
from concourse.dve_spec import (Spec, Src0, Src1, C0, C1, Zero, maxx, minn,
                                lower)
from concourse.dve_uop import DveOpSpec


def _make_elu_op():
    """out = relu(in0) + min(in1, c0) + c1  -- with c0=1, c1=-1 and
    in1=exp(in0) this is exactly elu(in0).  One DVE pass instead of a
    tensor_scalar + scalar_tensor_tensor pair."""
    name = "ELU_FUSED_ANT"
    for op in dve_ops.OPS:
        if op.name == name:
            return op
    spec = Spec(
        body=maxx(Src0, Zero) + minn(Src1, C0) + C1,
        reference=lambda in0, in1, c0, c1, c2: (
            np.maximum(in0.astype(np.float32), 0)
            + np.minimum(in1.astype(np.float32), c0) + c1),
    )
    idx = dve_ops._CUSTOM_DVE_ROW_BASE + len(dve_ops.OPS)
    shas = {}
    for ver in ("v3", "v4"):
        compiled = DveOpSpec(name=name, opcode=idx, uops=lower(spec, ver=ver),
                             rd1_en=True)
        shas[ver] = compiled.sha(ver)
    op = dve_ops.DveOp(name, spec, subdim=False, uops_sha=shas)
    dve_ops.OPS.append(op)
    dve_ops.CUSTOM_DVE_SPECS[name] = spec
    dve_ops._SUB_OPCODE_FOR_NAME[name] = idx
    return op


ELU_OP = _make_elu_op()

BF16 = ml_dtypes.bfloat16
F32 = mybir.dt.float32
F32R = mybir.dt.float32r
BF = mybir.dt.bfloat16
AF = mybir.ActivationFunctionType
ALU = mybir.AluOpType

B, M, D = 256, 128, 6
FA, FB, MSG, CONV = 128, 32, 128, 128
NCORES = 8
NMOL = B // NCORES           # molecules per core
NATOM = NMOL * M             # atoms per core (flat)

WARMUP_MMS = 0               # PE clock-ramp burst (measured: no effect)


def _roundup(x, m):
    return (x + m - 1) // m * m


def _chunks(caps):
    """h1 output chunks: (degree, start-within-bucket, width)."""
    out = []
    for d in range(D):
        cap = caps[d]
        for s0 in range(0, cap, 128):
            out.append((d, s0, min(128, cap - s0)))
    return out


# --------------------------------------------------------------------------
# device program
# --------------------------------------------------------------------------

def build_program(NA, caps, warmup=WARMUP_MMS):
    """SPMD Bass program. NA: active-atom grid size; caps: per-degree bucket
    sizes (sum == NA), uniform across all 8 cores."""
    assert sum(caps) == NA
    chunks = _chunks(caps)
    NCH = len(chunks)
    GW = 2 if 2 * NA <= 512 else 1       # degree blocks per matmul
    NG = D // GW

    nc = bacc.Bacc("TRN2", target_bir_lowering=False, debug=False,
                   enable_asserts=False, num_devices=NCORES)

    def din(name, shape):
        return nc.dram_tensor(name, list(shape), F32R,
                              kind="ExternalInput").ap()

    wmsg_d = din("wmsg", (128, 3, 128))     # w0a | w0b(pad) | w1
    nap_d = [din(f"nap{g}", (128, GW, NA)) for g in range(NG)]  # nbr groups
    bop_d = [din(f"bop{g}", (32, GW, NA)) for g in range(NG)]   # bond groups
    nact_d = din("nact", (128, NA))         # actT
    winn_d = din("winn", (128, 18, 128))    # iw0hi*6 | iw0lo*6 | iw1*6

    outp = nc.dram_tensor("outp", [NCH * 128, 128], F32,
                          kind="ExternalOutput")
    outp_ap = outp.ap()

    with tile.TileContext(nc) as tc:
        with (
            tc.tile_pool(name="w", bufs=1) as wp,
            tc.tile_pool(name="big", bufs=1) as bigp,
            tc.tile_pool(name="work", bufs=6) as work,
            tc.tile_pool(name="psM", bufs=3, space=bass.MemorySpace.PSUM) as psM,
            tc.tile_pool(name="psA", bufs=2, space=bass.MemorySpace.PSUM) as psA,
            tc.tile_pool(name="psW", bufs=1, space=bass.MemorySpace.PSUM) as psW,
        ):
            wmsg = wp.tile([128, 3, 128], F32R, tag="wmsg")
            nap = [wp.tile([128, GW, NA], F32R, tag=f"nap{g}", name=f"nap{g}")
                   for g in range(NG)]
            bop = [wp.tile([32, GW, NA], F32R, tag=f"bop{g}", name=f"bop{g}")
                   for g in range(NG)]
            nact = wp.tile([128, NA], F32R, tag="nact")
            winn = wp.tile([128, 18, 128], F32R, tag="winn")
            # need-order, alternating issue queues
            nc.sync.dma_start(wmsg[:], wmsg_d[:])
            for g in range(NG):
                nc.scalar.dma_start(nap[g][:], nap_d[g][:])
                nc.sync.dma_start(bop[g][:], bop_d[g][:])
            nc.scalar.dma_start(nact[:], nact_d[:])
            nc.sync.dma_start(winn[:], winn_d[:])

            w0a = wmsg[:, 0, :]
            w0b = wmsg[0:32, 1, :]
            w1 = wmsg[:, 2, :]

            def iw0hi(d):
                return winn[:, d, :]

            def iw0lo(d):
                return winn[:, 6 + d, :]

            def iw1(d):
                return winn[:, 12 + d, :]

            # ---- PE clock-ramp burst (no data deps; runs during DMA wait) --
            if warmup:
                wz = wp.tile([128, 256], BF, tag="wz")
                nc.vector.memset(wz[:], 0.0)
                pw = psW.tile([128, 512], F32, tag="psW")
                for _ in range(warmup):
                    nc.tensor.matmul(pw[:, 0:256], wz[:, 0:128], wz[:, 0:256],
                                     start=True, stop=True)

            # ---- inner0 for the largest bucket: the actT (lo) matmul has
            # no msg dependency, so run it right after the pre-matmuls and
            # let the hi-matmuls accumulate once the partial sums exist ----
            Sg = [0] * D
            acc = 0
            for d in range(D):
                Sg[d] = acc
                acc += caps[d]
            dbig = int(np.argmax(caps))
            capb = caps[dbig]
            pibig = psW.tile([128, 512], F32, tag="psW")

            # ---- message MLP, two degree blocks per matmul ----
            # All first-layer matmuls are emitted before any second-layer
            # matmul: the PE executes its queue in order, so a late msg1
            # matmul must not block the next group's independent pre-matmuls.
            assert NA * GW <= 512
            m1 = bigp.tile([128, 6, NA], F32R, tag="m1")
            pms = []
            for g in range(NG):
                pm = psM.tile([128, 512], F32, tag="pm")
                pv = pm[:, 0:GW * NA]
                nc.tensor.matmul(pv, w0a,
                                 nap[g][:].rearrange("p a b -> p (a b)"),
                                 start=True, stop=False)
                nc.tensor.matmul(pv, w0b,
                                 bop[g][:].rearrange("p a b -> p (a b)"),
                                 start=False, stop=True)
                pms.append(pv)
            if capb <= 512:
                nc.tensor.matmul(pibig[:, 0:capb], iw0lo(dbig),
                                 nact[:, Sg[dbig]:Sg[dbig] + capb],
                                 start=True, stop=False)
            m0s = []
            for g in range(NG):
                pv = pms[g]
                e0 = work.tile([128, GW * NA], F32R, tag="e0")
                m0 = work.tile([128, GW * NA], F32R, tag="m0")
                nc.scalar.activation(e0[:], pv, AF.Exp)
                nc.vector._custom_dve(ELU_OP, out=m0[:], in0=pv, in1=e0[:],
                                      s0=1.0, s1=-1.0)
                m0s.append(m0)
            pm2s = []
            for g in range(NG):
                pm2 = psM.tile([128, 512], F32, tag="pm")
                pv2 = pm2[:, 0:GW * NA]
                nc.tensor.matmul(pv2, w1, m0s[g][:], start=True, stop=True)
                pm2s.append(pv2)
            for g in range(NG):
                e1 = work.tile([128, GW * NA], F32R, tag="e0")
                nc.scalar.activation(e1[:], pm2s[g], AF.Exp)
                nc.vector._custom_dve(
                    ELU_OP,
                    out=m1[:, GW * g:GW * g + GW, :].rearrange(
                        "p a b -> p (a b)"),
                    in0=pm2s[g], in1=e1[:], s0=1.0, s1=-1.0)

            # ---- d-sum: 3 independent pair adds (each ready right after
            # its group); inner0 matmuls accumulate the three partials ----
            sp = [bigp.tile([128, NA], F32R, tag=f"sp{g}", name=f"sp{g}")
                  for g in range(3)]
            for g in range(3):
                nc.vector.tensor_tensor(sp[g][:], m1[:, 2 * g, :],
                                        m1[:, 2 * g + 1, :], ALU.add)
            del GW, NG

            # ---- per-degree inner MLP, layer 0 (largest bucket first) ----
            h0 = bigp.tile([128, NA], F32R, tag="h0")
            S = Sg
            order = sorted(range(D), key=lambda d: -caps[d])
            for d in order:
                cap = caps[d]
                if cap == 0:
                    continue
                off = S[d]
                for s0 in range(0, cap, 512):
                    w = min(512, cap - s0)
                    early = d == dbig and capb <= 512
                    if early:
                        pi = pibig
                    else:
                        pi = psA.tile([128, 512], F32, tag="psA")
                        nc.tensor.matmul(pi[:, 0:w], iw0lo(d),
                                         nact[:, off + s0:off + s0 + w],
                                         start=True, stop=False)
                    for g in range(3):
                        nc.tensor.matmul(pi[:, 0:w], iw0hi(d),
                                         sp[g][:, off + s0:off + s0 + w],
                                         start=False, stop=(g == 2))
                    eh = work.tile([128, 512], F32R, tag="eh")
                    nc.scalar.activation(eh[:, 0:w], pi[:, 0:w], AF.Exp)
                    nc.vector._custom_dve(
                        ELU_OP, out=h0[:, off + s0:off + s0 + w],
                        in0=pi[:, 0:w], in1=eh[:, 0:w], s0=1.0, s1=-1.0)

            # ---- inner layer 1 -> single chunk-major output DMA ----
            obuf = bigp.tile([128, NCH, 128], F32, tag="obuf")
            korder = sorted(range(NCH), key=lambda k: -chunks[k][2])
            for k in korder:
                d, s0, w = chunks[k]
                po = psA.tile([128, 512], F32, tag="psA")
                pov = po[0:w, 0:128]
                col = S[d] + s0
                nc.tensor.matmul(pov, h0[:, col:col + w], iw1(d),
                                 start=True, stop=True)
                eo = work.tile([128, 128], F32, tag="eo")
                nc.scalar.activation(eo[0:w, :], pov, AF.Exp)
                nc.vector._custom_dve(ELU_OP, out=obuf[0:w, k, :], in0=pov,
                                      in1=eo[0:w, :], s0=1.0, s1=-1.0)
                eng = nc.sync if k % 2 == 0 else nc.scalar
                eng.dma_start(outp_ap[k * 128:k * 128 + w, :],
                              obuf[0:w, k, :])

    nc.compile()
    return nc


_CACHE = {}


# --------------------------------------------------------------------------
# host side
# --------------------------------------------------------------------------

def _prep_core(atoms_c, bonds_c, edges_c, NA, caps):
    """Stage one core's arrays. Returns (dict name -> array, scatter info)."""
    af = atoms_c.reshape(NATOM, FA)
    bf = bonds_c.reshape(NATOM, D, FB)
    ef = edges_c.reshape(NATOM, D)
    deg = (ef != -1).sum(-1)

    act = np.nonzero(deg < D)[0]
    act = act[np.argsort(deg[act], kind="stable")]
    counts = np.bincount(deg[act], minlength=D)[:D]
    assert (counts <= np.asarray(caps)).all()

    S = np.concatenate([[0], np.cumsum(caps)])[:D]
    grid = np.full(NA, -1, np.int64)
    ofs = S.copy()
    for a in act:
        d = deg[a]
        grid[ofs[d]] = a
        ofs[d] += 1

    real = grid >= 0
    ga = grid[real]

    nbrT = np.zeros((128, D, NA), np.float32)
    e = ef[ga]
    mol = ga // M
    rcols = np.nonzero(real)[0]
    for d in range(D):
        has = e[:, d] >= 0
        nbrT[:, d, rcols[has]] = af[mol[has] * M + e[has, d]].T

    bo = np.zeros((32, D, NA), np.float32)
    bo[:, :, real] = bf[ga].transpose(2, 1, 0)
    nact = np.zeros((128, NA), np.float32)
    nact[:, real] = af[ga].T

    GW = 2 if 2 * NA <= 512 else 1
    m = dict(nact=nact)
    for g in range(D // GW):
        m[f"nap{g}"] = np.ascontiguousarray(nbrT[:, GW * g:GW * g + GW, :])
        m[f"bop{g}"] = np.ascontiguousarray(bo[:, GW * g:GW * g + GW, :])
    return m, ga, real


def _host_prep(atoms, bonds, edges):
    deg = (edges != -1).sum(-1).reshape(NCORES, NATOM)
    max_counts = np.zeros(D, np.int64)
    for c in range(NCORES):
        dc = deg[c]
        a = np.nonzero(dc < D)[0]
        cnt = np.bincount(dc[a], minlength=D)[:D]
        max_counts = np.maximum(max_counts, cnt)
    caps = [int(_roundup(x, 8)) if x > 0 else 0 for x in max_counts]
    NA = int(_roundup(max(sum(caps), 64), 16))
    caps[int(np.argmax(caps))] += NA - sum(caps)
    return NA, caps


def _pack_weights(msg_w0, msg_w1, inner_w0, inner_w1):
    wmsg = np.zeros((128, 3, 128), np.float32)
    wmsg[:, 0, :] = msg_w0[:128]
    wmsg[0:32, 1, :] = msg_w0[128:160]
    wmsg[:, 2, :] = msg_w1
    winn = np.zeros((128, 18, 128), np.float32)
    winn[:, 0:6, :] = inner_w0[:, :128, :].transpose(1, 0, 2)
    winn[:, 6:12, :] = inner_w0[:, 128:, :].transpose(1, 0, 2)
    winn[:, 12:18, :] = inner_w1.transpose(1, 0, 2)
    return wmsg, winn


def kernel(atoms, bonds, edges, msg_w0, msg_w1, inner_w0, inner_w1):
    atoms = np.asarray(atoms, np.float32)
    bonds = np.asarray(bonds, np.float32)
    edges = np.asarray(edges, np.int32)
    msg_w0 = np.asarray(msg_w0, np.float32)
    msg_w1 = np.asarray(msg_w1, np.float32)
    inner_w0 = np.asarray(inner_w0, np.float32)
    inner_w1 = np.asarray(inner_w1, np.float32)

    NA, caps = _host_prep(atoms, bonds, edges)

    key = (NA, tuple(caps))
    if key not in _CACHE:
        _CACHE[key] = build_program(NA, caps)
    nc = _CACHE[key]

    wmsg, winn = _pack_weights(msg_w0, msg_w1, inner_w0, inner_w1)

    in_maps = []
    scatter = []
    for c in range(NCORES):
        sl = slice(c * NMOL, (c + 1) * NMOL)
        m, ga, real = _prep_core(atoms[sl], bonds[sl], edges[sl], NA, caps)
        m["wmsg"] = wmsg
        m["winn"] = winn
        in_maps.append(m)
        scatter.append((ga, real))

    res = bass_utils.run_bass_kernel_spmd(
        nc, in_maps, core_ids=list(range(NCORES)))

    # unscatter: output rows are chunk-major (d, s0, w)
    chunks = _chunks(caps)
    S = np.concatenate([[0], np.cumsum(caps)])[:D]
    out = np.zeros((B * M, CONV), np.float32)
    for c in range(NCORES):
        ga, real = scatter[c]
        o = res.results[c]["outp"]
        full = np.zeros((NA, CONV), np.float32)
        for k, (d, s0, w) in enumerate(chunks):
            full[S[d] + s0:S[d] + s0 + w] = o[k * 128:k * 128 + w]
        out[c * NATOM + ga] = full[real]
    return out.reshape(B, M, CONV)



# revision 6
# speedup vs baseline: 1.2359x; 1.2359x over previous
"""Trainium2 Bass kernel for nn_NeuralGraphHidden (GNN message passing).

Structure: edges ~ randint(-1, 128) makes ~95.5% of atoms degree 6, whose
outputs are exactly zero (the reference's degree mask covers 0..5 only).  Of
the ~1440 "active" atoms, ~99% are degree 5.  The device handles ONLY the
degree-5 atoms (balanced across the 8 cores, NA~184/core); the handful of
degree<5 atoms are computed exactly on the host in numpy (microseconds).

Everything on device is bf16 (f32 PSUM accumulation): halves DMA vs f32,
LDWEIGHTS gets FWL (2x), and matmuls stream 1 col/cycle at any width.
Per-core device pipeline, with edge slots packed so the 5 real edges occupy
slots 0-4 and the padding slot's bond sits in slot 5 (nbr contribution zero):

  msg0_g  = elu(W0a.T @ nbrT_g + W0b.T @ bondT_g)    3 groups of 2 slots
  msg1_g  = elu(W1.T @ msg0_g)
  sp_j    = msg1 slot-pair sums (Pool adds)
  h0      = elu(iw0lo5.T @ actT + sum_j iw0hi5.T @ sp_j)   (PSUM accumulate)
  out     = elu(iw15.T @ h0)                          -> bf16 DMA out

elu(x) = relu(x) + min(exp(x), 1) - 1: exp on the ACT engine, the combine as
one fused custom-DVE op, except some tiles route the combine through the Pool
engine (tensor_scalar + add, max on DVE) to balance engine load.  An ACT-table
prewarm and a PE clock-ramp matmul burst run during the initial DMA wait.
"""

import sys

if "/opt/trn_rl_repo" not in sys.path:
    sys.path.insert(0, "/opt/trn_rl_repo")

import numpy as np
import ml_dtypes

import concourse.bass as bass
import concourse.bacc as bacc
import concourse.mybir as mybir
import concourse.tile as tile
from concourse import bass_utils

import concourse.dve_ops as dve_ops
from concourse.dve_spec import (Spec, Src0, Src1, C0, C1, Zero, maxx, minn,
                                lower)
from concourse.dve_uop import DveOpSpec


def _make_elu_op():
    """out = relu(in0) + min(in1, c0) + c1  -- with c0=1, c1=-1 and
    in1=exp(in0) this is exactly elu(in0)."""
    name = "ELU_FUSED_ANT"
    for op in dve_ops.OPS:
        if op.name == name:
            return op
    spec = Spec(
        body=maxx(Src0, Zero) + minn(Src1, C0) + C1,
        reference=lambda in0, in1, c0, c1, c2: (
            np.maximum(in0.astype(np.float32), 0)
            + np.minimum(in1.astype(np.float32), c0) + c1),
    )
    idx = dve_ops._CUSTOM_DVE_ROW_BASE + len(dve_ops.OPS)
    shas = {}
    for ver in ("v3", "v4"):
        compiled = DveOpSpec(name=name, opcode=idx, uops=lower(spec, ver=ver),
                             rd1_en=True)
        shas[ver] = compiled.sha(ver)
    op = dve_ops.DveOp(name, spec, subdim=False, uops_sha=shas)
    dve_ops.OPS.append(op)
    dve_ops.CUSTOM_DVE_SPECS[name] = spec
    dve_ops._SUB_OPCODE_FOR_NAME[name] = idx
    return op


ELU_OP = _make_elu_op()

BF16 = ml_dtypes.bfloat16
F32 = mybir.dt.float32
BF = mybir.dt.bfloat16
AF = mybir.ActivationFunctionType
ALU = mybir.AluOpType

B, M, D = 256, 128, 6
FA, FB, MSG, CONV = 128, 32, 128, 128
NCORES = 8

WARMUP_MMS = 7       # PE clock-ramp burst during the initial DMA wait


def _roundup(x, m):
    return (x + m - 1) // m * m


# --------------------------------------------------------------------------
# device program
# --------------------------------------------------------------------------

def build_program(NA, warmup=WARMUP_MMS):
    """SPMD program: NA degree-5 atom slots per core (multiple of 8)."""
    nc = bacc.Bacc("TRN2", target_bir_lowering=False, debug=False,
                   enable_asserts=False, num_devices=NCORES)

    # weights+nact fused: cols 0:128 w0a | 128:256 w0b(rows0-31) | 256:384 w1
    # | 384:512 hi5 | 512:640 lo5 | 640:768 iw15 | 768:768+NA nactT
    WS = 768 + NA
    wside_d = nc.dram_tensor("wside", [128, WS], BF, kind="ExternalInput").ap()
    nap_d = nc.dram_tensor("nap", [128, 5, NA], BF, kind="ExternalInput").ap()
    bop_d = nc.dram_tensor("bop", [32, 6, NA], BF, kind="ExternalInput").ap()
    outp = nc.dram_tensor("outp", [128, NA], BF, kind="ExternalOutput")
    outp_ap = outp.ap()

    H = NA // 2  # output half width

    with tile.TileContext(nc) as tc:
        with (
            tc.tile_pool(name="w", bufs=1) as wp,
            tc.tile_pool(name="work", bufs=8) as work,
            tc.tile_pool(name="psM", bufs=3, space=bass.MemorySpace.PSUM) as psM,
            tc.tile_pool(name="psA", bufs=2, space=bass.MemorySpace.PSUM) as psA,
        ):
            wside = wp.tile([128, WS], BF, tag="wside")
            nap = wp.tile([128, 5, NA], BF, tag="nap")
            bop = wp.tile([32, 6, NA], BF, tag="bop")

            # ---- input DMAs (need-order; only SP+ACT queues do HWDGE) ----
            nc.sync.dma_start(wside[:, 0:256], wside_d[:, 0:256])
            nc.scalar.dma_start(bop[:], bop_d[:])
            nc.sync.dma_start(nap[:, 0:2, :], nap_d[:, 0:2, :])
            nc.sync.dma_start(nap[:, 2:5, :], nap_d[:, 2:5, :])
            nc.sync.dma_start(wside[:, 256:WS], wside_d[:, 256:WS])

            w0a = wside[:, 0:128]
            w0b = wside[0:32, 128:256]
            w1 = wside[:, 256:384]
            hi5 = wside[:, 384:512]
            lo5 = wside[:, 512:640]
            iw15 = wside[:, 640:768]
            nact = wside[:, 768:WS]

            # ---- PE clock-ramp burst + ACT exp-table prewarm -------------
            wz = wp.tile([128, 512], BF, tag="wz")
            nc.vector.memset(wz[:], 0.0)
            escr = wp.tile([128, 1], F32, tag="escr")
            nc.scalar.activation(escr[:], wz[:, 0:1], AF.Exp)
            if warmup:
                pw = psA.tile([128, 512], F32, tag="ps")
                for _ in range(warmup):
                    nc.tensor.matmul(pw[:], wz[:, 0:128], wz[:],
                                     start=True, stop=True)

            # ---- msg layer 0: 3 groups of 2 edge slots -------------------
            bopv = bop[:].rearrange("p a b -> p (a b)")
            pms = []
            for g in range(3):
                pm = psM.tile([128, 2 * NA], F32, tag="pm")
                nc.tensor.matmul(pm[:], w0b, bopv[:, 2 * g * NA:(2 * g + 2) * NA],
                                 start=True, stop=False)
                pms.append(pm)
            for g in range(3):
                napv = nap[:, 2 * g:min(2 * g + 2, 5), :].rearrange(
                    "p a b -> p (a b)")
                w = 2 * NA if g < 2 else NA
                nc.tensor.matmul(pms[g][:, 0:w], w0a, napv,
                                 start=False, stop=True)

            # elu: exp on ACT, fused combine on DVE (GPSIMD can't read PSUM)
            def elu_tile(pv, out_ap, cols):
                """pv: PSUM f32 [128, cols]; out_ap: SBUF bf16 dest."""
                e = work.tile([128, cols], F32, tag=f"e{cols}")
                nc.scalar.activation(e[:], pv, AF.Exp)
                nc.vector._custom_dve(ELU_OP, out=out_ap, in0=pv,
                                      in1=e[:], s0=1.0, s1=-1.0)

            m0 = [work.tile([128, 2 * NA], BF, tag=f"m0_{g}", name=f"m0_{g}")
                  for g in range(3)]
            for g in range(3):
                elu_tile(pms[g][:], m0[g][:], 2 * NA)

            # ---- msg layer 1 --------------------------------------------
            m1 = wp.tile([128, 6, NA], BF, tag="m1")
            pm2s = []
            for g in range(3):
                pm2 = psM.tile([128, 2 * NA], F32, tag="pm")
                nc.tensor.matmul(pm2[:], w1, m0[g][:], start=True, stop=True)
                pm2s.append(pm2)
            for g in range(3):
                elu_tile(pm2s[g][:],
                         m1[:, 2 * g:2 * g + 2, :].rearrange("p a b -> p (a b)"),
                         2 * NA)

            # ---- slot-pair sums on Pool (3 adds) ------------------------
            sp = wp.tile([128, 3, NA], BF, tag="sp")
            for j in range(3):
                nc.gpsimd.tensor_tensor(sp[:, j, :], m1[:, 2 * j, :],
                                        m1[:, 2 * j + 1, :], ALU.add)

            # ---- inner layer 0 (degree-5 weights, PSUM accumulate) ------
            pi = psA.tile([128, NA], F32, tag="ps")
            nc.tensor.matmul(pi[:], lo5, nact, start=True, stop=False)
            for j in range(3):
                nc.tensor.matmul(pi[:], hi5, sp[:, j, :],
                                 start=False, stop=(j == 2))
            h0 = wp.tile([128, NA], BF, tag="h0")
            elu_tile(pi[:, 0:H], h0[:, 0:H], H)
            elu_tile(pi[:, H:NA], h0[:, H:NA], NA - H)

            # ---- inner layer 1 + output (two halves, two DMA queues) ----
            obuf = wp.tile([128, NA], BF, tag="obuf")
            po = psA.tile([128, NA], F32, tag="ps")
            nc.tensor.matmul(po[:, 0:H], iw15, h0[:, 0:H],
                             start=True, stop=True)
            elu_tile(po[:, 0:H], obuf[:, 0:H], H)
            nc.sync.dma_start(outp_ap[:, 0:H], obuf[:, 0:H])
            nc.tensor.matmul(po[:, H:NA], iw15, h0[:, H:NA],
                             start=True, stop=True)
            elu_tile(po[:, H:NA], obuf[:, H:NA], NA - H)
            nc.scalar.dma_start(outp_ap[:, H:NA], obuf[:, H:NA])

    nc.compile()
    return nc


_CACHE = {}


# --------------------------------------------------------------------------
# host side
# --------------------------------------------------------------------------

def _elu(x):
    return np.where(x > 0, x, np.expm1(np.minimum(x, 0.0)))


def _host_fallback(af, bf, ef, deg, ids, msg_w0, msg_w1, inner_w0, inner_w1):
    """Exact f32 reference for the (few) active atoms with degree < 5.
    af: (N,FA) atoms flat; bf: (N,D,FB); ef: (N,D); ids: flat atom indices."""
    if len(ids) == 0:
        return np.zeros((0, CONV), np.float32)
    mol = ids // M
    e = ef[ids]                                   # (n, D)
    nbr = np.where(e[..., None] >= 0,
                   af[(mol[:, None] * M + np.maximum(e, 0)).ravel()]
                   .reshape(len(ids), D, FA),
                   0.0)
    msg_in = np.concatenate([nbr, bf[ids]], axis=-1)        # (n, D, FA+FB)
    msg = _elu(msg_in @ msg_w0)
    msg = _elu(msg @ msg_w1)
    summed = msg.sum(axis=1)                                # (n, MSG)
    s2 = np.concatenate([summed, af[ids]], axis=-1)         # (n, MSG+FA)
    dg = deg[ids]
    h = _elu(np.einsum('nf,nfc->nc', s2, inner_w0[dg]))
    h = _elu(np.einsum('nc,nce->ne', h, inner_w1[dg]))
    return h.astype(np.float32)


def _prep_core(af, bf, ef, ids, NA):
    """Stage one core's deg-5 atoms (flat ids into af/bf/ef)."""
    n = len(ids)
    mol = ids // M
    e = ef[ids]                                   # (n, 6), exactly one -1
    real = e >= 0                                 # (n, 6) 5 True per row
    # pack real edges into slots 0-4, pad bond into slot 5
    order = np.argsort(~real, axis=1, kind="stable")   # real first
    e_p = np.take_along_axis(e, order, axis=1)         # (n,6) col5 = -1
    b_p = np.take_along_axis(bf[ids], order[..., None], axis=1)  # (n,6,FB)

    nap = np.zeros((128, 5, NA), BF16)
    src = af[(mol[:, None] * M + e_p[:, :5]).ravel()].reshape(n, 5, FA)
    nap[:, :, :n] = src.transpose(2, 1, 0).astype(BF16)
    bop = np.zeros((32, 6, NA), BF16)
    bop[:, :, :n] = b_p.transpose(2, 1, 0).astype(BF16)
    nact = np.zeros((128, NA), np.float32)
    nact[:, :n] = af[ids].T
    return nap, bop, nact


def _pack_wside(msg_w0, msg_w1, inner_w0, inner_w1, nact, NA):
    ws = np.zeros((128, 768 + NA), np.float32)
    ws[:, 0:128] = msg_w0[:128]
    ws[0:32, 128:256] = msg_w0[128:160]
    ws[:, 256:384] = msg_w1
    ws[:, 384:512] = inner_w0[5, :128, :]
    ws[:, 512:640] = inner_w0[5, 128:, :]
    ws[:, 640:768] = inner_w1[5]
    ws[:, 768:] = nact
    return ws.astype(BF16)


def kernel(atoms, bonds, edges, msg_w0, msg_w1, inner_w0, inner_w1):
    atoms = np.asarray(atoms, np.float32)
    bonds = np.asarray(bonds, np.float32)
    edges = np.asarray(edges, np.int32)
    msg_w0 = np.asarray(msg_w0, np.float32)
    msg_w1 = np.asarray(msg_w1, np.float32)
    inner_w0 = np.asarray(inner_w0, np.float32)
    inner_w1 = np.asarray(inner_w1, np.float32)

    af = atoms.reshape(B * M, FA)
    bf = bonds.reshape(B * M, D, FB)
    ef = edges.reshape(B * M, D)
    deg = (ef != -1).sum(-1)

    d5 = np.nonzero(deg == 5)[0]
    rest = np.nonzero(deg < 5)[0]

    # balanced round-robin assignment of deg-5 atoms to cores
    per_core = [d5[c::NCORES] for c in range(NCORES)]
    NA = max(16, _roundup(max(len(p) for p in per_core), 8))

    if NA not in _CACHE:
        _CACHE[NA] = build_program(NA)
    nc = _CACHE[NA]

    in_maps = []
    for c in range(NCORES):
        ids = per_core[c]
        nap, bop, nact = _prep_core(af, bf, ef, ids, NA)
        in_maps.append({
            "nap": nap, "bop": bop,
            "wside": _pack_wside(msg_w0, msg_w1, inner_w0, inner_w1,
                                 nact, NA),
        })

    res = bass_utils.run_bass_kernel_spmd(
        nc, in_maps, core_ids=list(range(NCORES)))

    out = np.zeros((B * M, CONV), np.float32)
    for c in range(NCORES):
        ids = per_core[c]
        o = np.asarray(res.results[c]["outp"]).astype(np.float32)  # (128, NA)
        out[ids] = o[:, :len(ids)].T
    out[rest] = _host_fallback(af, bf, ef, deg, rest,
                               msg_w0, msg_w1, inner_w0, inner_w1)
    return out.reshape(B, M, CONV)


# revision 10
# speedup vs baseline: 1.3065x; 1.0571x over previous
"""Trainium2 Bass kernel for nn_NeuralGraphHidden (GNN message passing).

Structure: edges ~ randint(-1, 128) makes ~95.5% of atoms degree 6, whose
outputs are exactly zero (the reference's degree mask covers 0..5 only).  Of
the ~1440 "active" atoms, ~99% are degree 5.  The device handles ONLY the
degree-5 atoms (balanced across the 8 cores, NA~184/core); the handful of
degree<5 atoms are computed exactly on the host in numpy (microseconds).

Everything on device is bf16 (f32 PSUM accumulation): halves DMA vs f32,
LDWEIGHTS gets FWL (2x), and matmuls stream 1 col/cycle at any width.
Per-core device pipeline, with edge slots packed so the 5 real edges occupy
slots 0-4 and the padding slot's bond sits in slot 5 (nbr contribution zero):

  msg0_g  = elu(W0a.T @ nbrT_g + W0b.T @ bondT_g)    3 groups of 2 slots
  msg1_g  = elu(W1.T @ msg0_g)
  h0      = elu(iw0lo5.T @ actT + sum_j iw0hi5.T @ msg1_j)  (PSUM accumulate)
  out     = elu(iw15.T @ h0)                          -> bf16 DMA out

elu(x) = relu(x) + min(exp(x), 1) - 1: exp on the ACT engine (bf16 out), the
combine as one fused custom-DVE op.  An ACT-table prewarm and a PE clock-ramp
matmul burst run during the initial DMA wait.
"""

import sys

if "/opt/trn_rl_repo" not in sys.path:
    sys.path.insert(0, "/opt/trn_rl_repo")

import numpy as np
import ml_dtypes

import concourse.bass as bass
import concourse.bacc as bacc
import concourse.mybir as mybir
import concourse.tile as tile
from concourse import bass_utils

import concourse.dve_ops as dve_ops
from concourse.dve_spec import (Spec, Src0, Src1, C0, C1, Zero, maxx, minn,
                                lower)
from concourse.dve_uop import DveOpSpec


def _make_elu_op():
    """out = relu(in0) + min(in1, c0) + c1  -- with c0=1, c1=-1 and
    in1=exp(in0) this is exactly elu(in0)."""
    name = "ELU_FUSED_ANT"
    for op in dve_ops.OPS:
        if op.name == name:
            return op
    spec = Spec(
        body=maxx(Src0, Zero) + minn(Src1, C0) + C1,
        reference=lambda in0, in1, c0, c1, c2: (
            np.maximum(in0.astype(np.float32), 0)
            + np.minimum(in1.astype(np.float32), c0) + c1),
    )
    idx = dve_ops._CUSTOM_DVE_ROW_BASE + len(dve_ops.OPS)
    shas = {}
    for ver in ("v3", "v4"):
        compiled = DveOpSpec(name=name, opcode=idx, uops=lower(spec, ver=ver),
                             rd1_en=True)
        shas[ver] = compiled.sha(ver)
    op = dve_ops.DveOp(name, spec, subdim=False, uops_sha=shas)
    dve_ops.OPS.append(op)
    dve_ops.CUSTOM_DVE_SPECS[name] = spec
    dve_ops._SUB_OPCODE_FOR_NAME[name] = idx
    return op


ELU_OP = _make_elu_op()

BF16 = ml_dtypes.bfloat16
F32 = mybir.dt.float32
BF = mybir.dt.bfloat16
AF = mybir.ActivationFunctionType
ALU = mybir.AluOpType

B, M, D = 256, 128, 6
FA, FB, MSG, CONV = 128, 32, 128, 128
NCORES = 8

WARMUP_MMS = 4       # PE clock-ramp burst during the initial DMA wait


def _roundup(x, m):
    return (x + m - 1) // m * m


# --------------------------------------------------------------------------
# device program
# --------------------------------------------------------------------------

def build_program(NA, warmup=WARMUP_MMS):
    """SPMD program: NA degree-5 atom slots per core (multiple of 8)."""
    nc = bacc.Bacc("TRN2", target_bir_lowering=False, debug=False,
                   enable_asserts=False, num_devices=NCORES)

    # weights+nact fused: cols 0:128 w0a | 128:256 w0b(rows0-31) | 256:384 w1
    # | 384:512 hi5 | 512:640 lo5 | 640:768 iw15 | 768:768+NA nactT
    WS = 768 + NA
    wside_d = nc.dram_tensor("wside", [128, WS], BF, kind="ExternalInput").ap()
    nap_d = nc.dram_tensor("nap", [128, 5, NA], BF, kind="ExternalInput").ap()
    bop_d = nc.dram_tensor("bop", [32, 6, NA], BF, kind="ExternalInput").ap()
    outp = nc.dram_tensor("outp", [128, NA], BF, kind="ExternalOutput")
    outp_ap = outp.ap()

    H = NA // 2  # output half width

    with tile.TileContext(nc) as tc:
        with (
            tc.tile_pool(name="w", bufs=1) as wp,
            tc.tile_pool(name="work", bufs=8) as work,
            tc.tile_pool(name="psM", bufs=3, space=bass.MemorySpace.PSUM) as psM,
            tc.tile_pool(name="psA", bufs=2, space=bass.MemorySpace.PSUM) as psA,
        ):
            wside = wp.tile([128, WS], BF, tag="wside")
            nap = wp.tile([128, 5, NA], BF, tag="nap")
            bop = wp.tile([32, 6, NA], BF, tag="bop")

            # ---- input DMAs (need-order; only SP+ACT queues do HWDGE) ----
            nc.sync.dma_start(wside[:, 0:384], wside_d[:, 0:384])
            nc.scalar.dma_start(bop[:, 0:2, :], bop_d[:, 0:2, :])
            nc.sync.dma_start(nap[:, 0:2, :], nap_d[:, 0:2, :])
            nc.scalar.dma_start(bop[:, 2:6, :], bop_d[:, 2:6, :])
            nc.sync.dma_start(nap[:, 2:5, :], nap_d[:, 2:5, :])
            nc.sync.dma_start(wside[:, 384:WS], wside_d[:, 384:WS])

            w0a = wside[:, 0:128]
            w0b = wside[0:32, 128:256]
            w1 = wside[:, 256:384]
            hi5 = wside[:, 384:512]
            lo5 = wside[:, 512:640]
            iw15 = wside[:, 640:768]
            nact = wside[:, 768:WS]

            # ---- PE clock-ramp burst + ACT exp-table prewarm -------------
            wz = wp.tile([128, 512], BF, tag="wz")
            nc.vector.memset(wz[:], 0.0)
            escr = wp.tile([128, 1], F32, tag="escr")
            nc.scalar.activation(escr[:], wz[:, 0:1], AF.Exp)
            if warmup:
                pw = psA.tile([128, 512], F32, tag="ps")
                for _ in range(warmup):
                    nc.tensor.matmul(pw[:], wz[:, 0:128], wz[:],
                                     start=True, stop=True)

            # ---- msg layer 0: 3 groups of 2 edge slots -------------------
            bopv = bop[:].rearrange("p a b -> p (a b)")
            pms = []
            for g in range(3):
                pm = psM.tile([128, 2 * NA], F32, tag="pm")
                nc.tensor.matmul(pm[:], w0b, bopv[:, 2 * g * NA:(2 * g + 2) * NA],
                                 start=True, stop=False)
                pms.append(pm)
            for g in range(3):
                napv = nap[:, 2 * g:min(2 * g + 2, 5), :].rearrange(
                    "p a b -> p (a b)")
                w = 2 * NA if g < 2 else NA
                nc.tensor.matmul(pms[g][:, 0:w], w0a, napv,
                                 start=False, stop=True)

            # elu: exp on ACT, fused combine on DVE (GPSIMD can't read PSUM)
            def elu_tile(pv, out_ap, cols):
                """pv: PSUM f32 [128, cols]; out_ap: SBUF bf16 dest."""
                e = work.tile([128, cols], BF, tag=f"e{cols}")
                nc.scalar.activation(e[:], pv, AF.Exp)
                nc.vector._custom_dve(ELU_OP, out=out_ap, in0=pv,
                                      in1=e[:], s0=1.0, s1=-1.0)

            m0 = [work.tile([128, 2 * NA], BF, tag=f"m0_{g}", name=f"m0_{g}")
                  for g in range(3)]
            for g in range(3):
                elu_tile(pms[g][:], m0[g][:], 2 * NA)

            # ---- msg layer 1 --------------------------------------------
            m1 = wp.tile([128, 6, NA], BF, tag="m1")
            pm2s = []
            for g in range(3):
                pm2 = psM.tile([128, 2 * NA], F32, tag="pm")
                nc.tensor.matmul(pm2[:], w1, m0[g][:], start=True, stop=True)
                pm2s.append(pm2)
            for g in range(3):
                elu_tile(pm2s[g][:],
                         m1[:, 2 * g:2 * g + 2, :].rearrange("p a b -> p (a b)"),
                         2 * NA)

            # ---- inner layer 0 (degree-5 weights, PSUM accumulate) ------
            pi = psA.tile([128, NA], F32, tag="ps")
            nc.tensor.matmul(pi[:], lo5, nact, start=True, stop=False)
            for j in range(6):
                nc.tensor.matmul(pi[:], hi5, m1[:, j, :],
                                 start=False, stop=(j == 5))
            h0 = wp.tile([128, NA], BF, tag="h0")
            elu_tile(pi[:, 0:H], h0[:, 0:H], H)
            elu_tile(pi[:, H:NA], h0[:, H:NA], NA - H)

            # ---- inner layer 1 + output (two halves, two DMA queues) ----
            obuf = wp.tile([128, NA], BF, tag="obuf")
            po = psA.tile([128, NA], F32, tag="ps")
            nc.tensor.matmul(po[:, 0:H], iw15, h0[:, 0:H],
                             start=True, stop=True)
            elu_tile(po[:, 0:H], obuf[:, 0:H], H)
            nc.sync.dma_start(outp_ap[:, 0:H], obuf[:, 0:H])
            nc.tensor.matmul(po[:, H:NA], iw15, h0[:, H:NA],
                             start=True, stop=True)
            elu_tile(po[:, H:NA], obuf[:, H:NA], NA - H)
            nc.scalar.dma_start(outp_ap[:, H:NA], obuf[:, H:NA])

    nc.compile()
    return nc


_CACHE = {}


# --------------------------------------------------------------------------
# host side
# --------------------------------------------------------------------------

def _elu(x):
    return np.where(x > 0, x, np.expm1(np.minimum(x, 0.0)))


def _host_fallback(af, bf, ef, deg, ids, msg_w0, msg_w1, inner_w0, inner_w1):
    """Exact f32 reference for the (few) active atoms with degree < 5.
    af: (N,FA) atoms flat; bf: (N,D,FB); ef: (N,D); ids: flat atom indices."""
    if len(ids) == 0:
        return np.zeros((0, CONV), np.float32)
    mol = ids // M
    e = ef[ids]                                   # (n, D)
    nbr = np.where(e[..., None] >= 0,
                   af[(mol[:, None] * M + np.maximum(e, 0)).ravel()]
                   .reshape(len(ids), D, FA),
                   0.0)
    msg_in = np.concatenate([nbr, bf[ids]], axis=-1)        # (n, D, FA+FB)
    msg = _elu(msg_in @ msg_w0)
    msg = _elu(msg @ msg_w1)
    summed = msg.sum(axis=1)                                # (n, MSG)
    s2 = np.concatenate([summed, af[ids]], axis=-1)         # (n, MSG+FA)
    dg = deg[ids]
    h = _elu(np.einsum('nf,nfc->nc', s2, inner_w0[dg]))
    h = _elu(np.einsum('nc,nce->ne', h, inner_w1[dg]))
    return h.astype(np.float32)


def _prep_core(af, bf, ef, ids, NA):
    """Stage one core's deg-5 atoms (flat ids into af/bf/ef)."""
    n = len(ids)
    mol = ids // M
    e = ef[ids]                                   # (n, 6), exactly one -1
    real = e >= 0                                 # (n, 6) 5 True per row
    # pack real edges into slots 0-4, pad bond into slot 5
    order = np.argsort(~real, axis=1, kind="stable")   # real first
    e_p = np.take_along_axis(e, order, axis=1)         # (n,6) col5 = -1
    b_p = np.take_along_axis(bf[ids], order[..., None], axis=1)  # (n,6,FB)

    nap = np.zeros((128, 5, NA), BF16)
    src = af[(mol[:, None] * M + e_p[:, :5]).ravel()].reshape(n, 5, FA)
    nap[:, :, :n] = src.transpose(2, 1, 0).astype(BF16)
    bop = np.zeros((32, 6, NA), BF16)
    bop[:, :, :n] = b_p.transpose(2, 1, 0).astype(BF16)
    nact = np.zeros((128, NA), np.float32)
    nact[:, :n] = af[ids].T
    return nap, bop, nact


def _pack_wside(msg_w0, msg_w1, inner_w0, inner_w1, nact, NA):
    ws = np.zeros((128, 768 + NA), np.float32)
    ws[:, 0:128] = msg_w0[:128]
    ws[0:32, 128:256] = msg_w0[128:160]
    ws[:, 256:384] = msg_w1
    ws[:, 384:512] = inner_w0[5, :128, :]
    ws[:, 512:640] = inner_w0[5, 128:, :]
    ws[:, 640:768] = inner_w1[5]
    ws[:, 768:] = nact
    return ws.astype(BF16)


def kernel(atoms, bonds, edges, msg_w0, msg_w1, inner_w0, inner_w1):
    atoms = np.asarray(atoms, np.float32)
    bonds = np.asarray(bonds, np.float32)
    edges = np.asarray(edges, np.int32)
    msg_w0 = np.asarray(msg_w0, np.float32)
    msg_w1 = np.asarray(msg_w1, np.float32)
    inner_w0 = np.asarray(inner_w0, np.float32)
    inner_w1 = np.asarray(inner_w1, np.float32)

    af = atoms.reshape(B * M, FA)
    bf = bonds.reshape(B * M, D, FB)
    ef = edges.reshape(B * M, D)
    deg = (ef != -1).sum(-1)

    d5 = np.nonzero(deg == 5)[0]
    rest = np.nonzero(deg < 5)[0]

    # balanced round-robin assignment of deg-5 atoms to cores
    per_core = [d5[c::NCORES] for c in range(NCORES)]
    NA = max(16, _roundup(max(len(p) for p in per_core), 8))

    if NA not in _CACHE:
        _CACHE[NA] = build_program(NA)
    nc = _CACHE[NA]

    in_maps = []
    for c in range(NCORES):
        ids = per_core[c]
        nap, bop, nact = _prep_core(af, bf, ef, ids, NA)
        in_maps.append({
            "nap": nap, "bop": bop,
            "wside": _pack_wside(msg_w0, msg_w1, inner_w0, inner_w1,
                                 nact, NA),
        })

    res = bass_utils.run_bass_kernel_spmd(
        nc, in_maps, core_ids=list(range(NCORES)))

    out = np.zeros((B * M, CONV), np.float32)
    for c in range(NCORES):
        ids = per_core[c]
        o = np.asarray(res.results[c]["outp"]).astype(np.float32)  # (128, NA)
        out[ids] = o[:, :len(ids)].T
    out[rest] = _host_fallback(af, bf, ef, deg, rest,
                               msg_w0, msg_w1, inner_w0, inner_w1)
    return out.reshape(B, M, CONV)
